# revision 22
# baseline (speedup 1.0000x reference)
"""Trainium2 Bass kernel for nn_D_GA_1812476199112 (maxpool -> 16-head
attention over 1024 tokens -> proj -> batchnorm -> maxunpool).

Sharding: data-parallel over batch B=8, one batch element per NeuronCore.
Everything is local per core; no collectives.

Per-core pipeline (channels-on-partitions layout [C=64, N=1024]):
  1. MaxPool2d(2,2) via strided max ops pipelined with the x DMA in four
     quarters (DVE+Pool); argmax becomes first-match masks computed
     per-quarter on GpSimd during attention idle.
  2. Q^T/K^T strip-packed (head h=4sg+c at partitions 32c..32c+4) from
     host-permuted padded weights, laid out quarter-major so one copy
     drains two supergroups; pooled is f32r so prep runs single-pass.
     Prep flows through the same PSUM slot rotation as score chunks.
  3. Score matmuls compute S^T [keys, queries] (K=4, row strips) into
     [128, 1024] PSUM slots (3-slot rotation), 4 blocks of 256 queries.
  4. Softmax exp splits across two engines: ACT runs real Exp (scale 0.5
     folded); DVE runs a Schraudolph bit-trick exp (i32 = A*s + B written
     through an int32 bitcast of the f32r e-tile; one tensor_scalar per
     chunk; end-to-end error ~2e-3, verified bit-exact vs numpy on HW).
  5. AV uses the EXP TILE AS THE STATIONARY operand: out[q,5] accumulates
     e_chunk[keys, qb*128..]^T @ v~[keys, 5h] over key tiles (ones col of
     v~ gives softmax denominators for free).  5-col bf16-moving matmuls
     cost ~2ns each, vs 131k PE columns for the moving-e formulation.
     AV lags the slot rotation by the slot count so it never stalls PE.
  6. Work runs in 4 query-quarter passes (256 q each) so the tail
     (normalize, PE transpose via identity, proj, BN Identity, unpool,
     DMA out) of each quarter hides under the next quarter's attention.
"""
import numpy as np

DIM = 64
HEAD_DIM = 4
NUM_HEADS = 16
B = 8
H = W = 64
HP = WP = 32
N = HP * WP          # 1024 tokens
NKT = 8              # key tiles of 128
QW = 256             # queries per quarter-run
BN_EPS = 1e-5

# Schraudolph exp constants (folds the softmax scale 0.5):
#   e^(0.5*s) ~= bitcast_f32(int32(A_EXP*s + B_EXP))
A_EXP = float(0.5 * np.log2(np.e) * (1 << 23))
B_EXP = float(127 * (1 << 23) - 486411)

_CACHE = {}

# ---- tuning knobs ----
N_WARM = 12          # PE warmup dummies at t=0
N_WARM2 = 6          # bridge dummies before prep
AV_LAG = 4           # chunks between scores and their AV flush (= slot count)
ACT_FRAC = 0.54      # fraction of exp chunks on ACT (rest DVE)
VT_ON_ACT = True     # v~ PSUM->bf16 copies on ACT instead of DVE


def _build_program(debug_taps=False):
    import concourse.bass as bass
    import concourse.mybir as mybir
    import concourse.tile as tile
    from concourse import bacc

    f32 = mybir.dt.float32
    f32r = mybir.dt.float32r
    bf16 = mybir.dt.bfloat16
    i32 = mybir.dt.int32
    AF = mybir.ActivationFunctionType
    OP = mybir.AluOpType

    nc = bacc.Bacc("TRN2", debug=False)

    x_d = nc.dram_tensor("x", [DIM, H * W], f32, kind="ExternalInput").ap()
    wb_d = nc.dram_tensor("wb", [128, 322], f32, kind="ExternalInput").ap()
    wr_d = nc.dram_tensor("wr", [64, 64], f32r, kind="ExternalInput").ap()
    out_d = nc.dram_tensor("out", [DIM, H * W], f32, kind="ExternalOutput").ap()

    with tile.TileContext(nc) as tc:
        with (
            tc.tile_pool(name="singles", bufs=1) as sg1,
            tc.tile_pool(name="expp", bufs=5) as expp,
            tc.tile_pool(name="slots", bufs=3, space="PSUM") as slots,
            tc.tile_pool(name="accp", bufs=1, space="PSUM") as accp,
        ):
            # hoist the ACT exp-table load to t=0 via a dummy exp
            warm = sg1.tile([1, 1], f32)
            nc.vector.memset(warm, 0.0)
            nc.scalar.activation(warm, warm, AF.Exp)

            # ---------- loads ----------
            x_sb = sg1.tile([DIM, H * W], f32)
            xr = x_sb.rearrange("p (i ti j tj) -> p i ti j tj", ti=2, tj=2, j=WP)
            wb_sb = sg1.tile([128, 322], f32)
            wr_sb = sg1.tile([64, 64], f32r)
            nc.sync.dma_start(out=wr_sb, in_=wr_d)
            nc.sync.dma_start(out=x_sb[:, 0:1024], in_=x_d[:, 0:1024])
            nc.sync.dma_start(out=wb_sb, in_=wb_d)
            for qq in (1, 2, 3):
                nc.sync.dma_start(out=x_sb[:, qq * 1024:(qq + 1) * 1024],
                                  in_=x_d[:, qq * 1024:(qq + 1) * 1024])
            wproj_sb = wr_sb
            bns_sb = wb_sb[0:64, 128:129]
            bnb_sb = wb_sb[0:64, 129:130]
            wv_sb = wb_sb[0:64, 130:194]                    # [64, 64] f32
            ident_sb = wb_sb[:, 194:322]                    # [128, 128] f32

            # strip-pack dense wq/wk on device: head h=4sg+c lives at cols
            # 32c..32c+4 of qkp's per-sg 128-col block (q), +512 for k
            qkp = sg1.tile([64, 1024], f32)  # plain f32: prep runs 4-pass
            nc.gpsimd.memset(qkp, 0.0)
            for sg in range(4):
                for qk in range(2):
                    dst = qkp[:, qk * 512 + sg * 128:qk * 512 + (sg + 1) * 128]
                    nc.gpsimd.tensor_copy(
                        dst.rearrange("p (c x d) -> p c x d", c=4, x=8)[:, :, 0, 0:4],
                        wb_sb[0:64, 64 * qk + 16 * sg:64 * qk + 16 * (sg + 1)]
                        .rearrange("p (c d) -> p c d", d=4))
            wqp_sb = [qkp[:, 128 * sg:128 * sg + 128] for sg in range(4)]
            wkp_sb = [qkp[:, 512 + 128 * sg:512 + 128 * sg + 128]
                      for sg in range(4)]

            # PE warmup dummies (keep HAM clock hot through the head phase)
            dummy_bf = sg1.tile([64, 512], bf16)
            nc.vector.memset(dummy_bf, 1.0)

            def emit_dummies(k):
                for _ in range(k):
                    d_ps = slots.tile([128, 512], f32, tag="slotB", bufs=1,
                                      name="d_ps")
                    nc.tensor.matmul(d_ps[0:64, :], dummy_bf[:, 0:64], dummy_bf,
                                     start=True, stop=True)

            emit_dummies(N_WARM)

            # ---------- maxpool (per x-quarter) ----------
            pooled = sg1.tile([DIM, N], f32)
            m01 = sg1.tile([DIM, N], f32)
            m23 = sg1.tile([DIM, N], f32)
            m01r = m01.rearrange("p (i j) -> p i j", j=WP)
            m23r = m23.rearrange("p (i j) -> p i j", j=WP)
            pooledr = pooled.rearrange("p (i j) -> p i j", j=WP)
            v = [xr[:, :, 0, :, 0], xr[:, :, 0, :, 1],
                 xr[:, :, 1, :, 0], xr[:, :, 1, :, 1]]

            def emit_maxpool_quarter(qq, pool_only=False):
                sl = slice(qq * 8, (qq + 1) * 8)
                e1 = nc.gpsimd if pool_only else nc.vector
                nc.gpsimd.tensor_tensor(m23r[:, sl], v[2][:, sl], v[3][:, sl],
                                        op=OP.max)
                e1.tensor_tensor(m01r[:, sl], v[0][:, sl], v[1][:, sl], op=OP.max)
                e1.tensor_tensor(pooledr[:, sl], m01r[:, sl], m23r[:, sl],
                                 op=OP.max)

            emit_maxpool_quarter(0)
            emit_maxpool_quarter(1)
            emit_dummies(N_WARM2)

            # ---------- persistent SBUF tiles ----------
            # qk layout (bf16): [128, (qq)(pair)(q512|k512)] where the 512
            # is 2 supergroups x 256 quarter-queries/keys
            qk = sg1.tile([128, 8192], bf16)
            vt = [sg1.tile([128, NUM_HEADS, 5], bf16, tag=f"vt{kt}", name=f"vt{kt}")
                  for kt in range(NKT)]
            ones16 = sg1.tile([128, 16], bf16)
            nc.vector.memset(ones16, 1.0)
            o_norm = sg1.tile([128, 128], f32)     # [q2, h16, d4]
            o_normT = sg1.tile([64, N], f32r)
            dr = sg1.tile([128, 32], f32)
            y = sg1.tile([DIM, N], f32)
            yr = y.rearrange("p (i j) -> p i j", j=WP)
            out_sb = sg1.tile([DIM, H * W], f32)
            outr = out_sb.rearrange("p (i ti j tj) -> p i ti j tj",
                                    ti=2, tj=2, j=WP)

            def q_slice(sg, qs):
                base = ((qs // 256) * 2048 + (sg // 2) * 1024
                        + (sg % 2) * 256 + ((qs // 128) % 2) * 128)
                return slice(base, base + 128)

            def k_slice(sg, kt):
                base = ((kt // 2) * 2048 + (sg // 2) * 1024 + 512
                        + (sg % 2) * 256 + (kt % 2) * 128)
                return slice(base, base + 128)

            # ---------- masks (GpSimd filler, per i-quarter chains) --------
            eq = [sg1.tile([DIM, N], f32, tag=f"eq{p}", name=f"eq{p}")
                  for p in range(4)]
            nf = [sg1.tile([DIM, N], f32, tag=f"nf{p}", name=f"nf{p}")
                  for p in range(2)]
            eqr = [t.rearrange("p (i j) -> p i j", j=WP) for t in eq]
            nfr = [t.rearrange("p (i j) -> p i j", j=WP) for t in nf]
            mask_steps = []
            for mq in range(4):
                msl = slice(mq * 8, (mq + 1) * 8)
                g = nc.gpsimd
                mask_steps += [
                    lambda sl=msl, g=g: g.tensor_tensor(
                        eqr[0][:, sl], v[0][:, sl], pooledr[:, sl], op=OP.is_equal),
                    lambda sl=msl, g=g: g.tensor_tensor(
                        eqr[1][:, sl], v[1][:, sl], pooledr[:, sl], op=OP.is_equal),
                    lambda sl=msl, g=g: g.tensor_tensor(
                        eqr[2][:, sl], v[2][:, sl], pooledr[:, sl], op=OP.is_equal),
                    lambda sl=msl, g=g: g.tensor_tensor(
                        eqr[3][:, sl], v[3][:, sl], pooledr[:, sl], op=OP.is_equal),
                    lambda sl=msl, g=g: g.tensor_scalar(
                        nfr[0][:, sl], eqr[0][:, sl], -1.0, 1.0,
                        op0=OP.mult, op1=OP.add),
                    lambda sl=msl, g=g: g.tensor_tensor(
                        eqr[1][:, sl], eqr[1][:, sl], nfr[0][:, sl], op=OP.mult),
                    lambda sl=msl, g=g: g.tensor_tensor(
                        nfr[1][:, sl], nfr[0][:, sl], eqr[1][:, sl], op=OP.subtract),
                    lambda sl=msl, g=g: g.tensor_tensor(
                        eqr[2][:, sl], eqr[2][:, sl], nfr[1][:, sl], op=OP.mult),
                    lambda sl=msl, g=g: g.tensor_tensor(
                        nfr[0][:, sl], nfr[1][:, sl], eqr[2][:, sl], op=OP.subtract),
                    lambda sl=msl, g=g: g.tensor_tensor(
                        eqr[3][:, sl], eqr[3][:, sl], nfr[0][:, sl], op=OP.mult),
                ]
            mask_done = [0]

            def emit_mask_step():
                if mask_done[0] >= len(mask_steps):
                    return False
                mask_steps[mask_done[0]]()
                mask_done[0] += 1
                return True

            # ---------- pipelined work items over the PSUM slot rotation ----
            pend = []
            av_count = {}
            acc = [None] * 5
            est = {"act": 0.0, "dve": 0.0}

            def track(engine, ns):
                est[engine] += ns

            def emit_prep_item(pair, qq):
                """qt+kt prep for supergroups {2*pair, 2*pair+1}, quarter qq;
                one merged bf16 copy drains the slot."""
                psl = slice(qq * QW, (qq + 1) * QW)
                slot = slots.tile([128, 1024], f32, tag="slotA", bufs=3,
                                  name="slotP")
                for j, sg in enumerate((2 * pair, 2 * pair + 1)):
                    nc.tensor.matmul(slot[:, j * QW:(j + 1) * QW],
                                     wqp_sb[sg], pooled[:, psl],
                                     start=True, stop=True)
                    nc.tensor.matmul(slot[:, 512 + j * QW:512 + (j + 1) * QW],
                                     wkp_sb[sg], pooled[:, psl],
                                     start=True, stop=True)
                base = qq * 2048 + pair * 1024
                if est["act"] + 1040 <= est["dve"] + 1190:
                    nc.scalar.copy(qk[:, base:base + 1024], slot)
                    track("act", 1040)
                else:
                    nc.vector.tensor_copy(qk[:, base:base + 1024], slot)
                    track("dve", 1190)

            def emit_v_item(qq):
                """V~ for key tiles 2*qq, 2*qq+1 via the small PSUM slot."""
                v_ps = slots.tile([128, 512], f32, tag="slotB", bufs=1,
                                  name="v_ps")
                for j in range(2):
                    kt = 2 * qq + j
                    nc.tensor.matmul(v_ps[:, j * 64:(j + 1) * 64],
                                     pooled[:, kt * 128:(kt + 1) * 128], wv_sb,
                                     start=True, stop=True)
                cp = nc.scalar.copy if VT_ON_ACT else nc.vector.tensor_copy
                track("act" if VT_ON_ACT else "dve", 520)
                for j in range(2):
                    kt = 2 * qq + j
                    cp(vt[kt][:, :, 0:4],
                       v_ps[:, j * 64:(j + 1) * 64].rearrange(
                           "p (h e) -> p h e", e=4))
                    nc.gpsimd.tensor_copy(
                        vt[kt].rearrange("p h e -> p (h e)")[:, 4::5], ones16)

            def flush_av(payload):
                ch, e_sb, run_idx, run_start = payload
                for i, (h, kt, qs) in enumerate(ch):
                    qbl = (qs - run_start) // 128
                    key = (qs, h)
                    cnt = av_count.get(key, 0)
                    av_count[key] = cnt + 1
                    nc.tensor.matmul(
                        acc[run_idx][:, qbl * 128 + h * 8:qbl * 128 + h * 8 + 5],
                        e_sb[:, i * 128:(i + 1) * 128],
                        vt[kt][:, h, :],
                        start=False, stop=(cnt == NKT - 1),
                        skip_group_check=True)

            chunk_idx = [0]

            def emit_score_chunk(ch, run_idx, run_start, engine=None):
                ncb = len(ch)
                svc_a = 107 * ncb + 220
                svc_d = 133 * ncb + 160
                if engine is None:
                    engine = ("act" if est["act"] + svc_a <= est["dve"] + svc_d
                              else "dve")
                if ncb > 4:
                    slot = slots.tile([128, 1024], f32, tag="slotA", bufs=3,
                                      name="slotA")
                else:
                    slot = slots.tile([128, 512], f32, tag="slotB", bufs=1,
                                      name="slotB")
                for i, (h, kt, qs) in enumerate(ch):
                    sg, c = h // 4, h % 4
                    nc.tensor.matmul(
                        slot[:, i * 128:(i + 1) * 128],
                        qk[32 * c:32 * c + 4, k_slice(sg, kt)],
                        qk[32 * c:32 * c + 4, q_slice(sg, qs)],
                        start=True, stop=True,
                        tile_position=(32 * c, 0))
                if ncb > 4:
                    e_sb = expp.tile([128, 1024], f32r, tag="exp", bufs=5,
                                     name="e_sb")
                else:
                    e_sb = expp.tile([128, 512], f32r, tag="expB", bufs=2,
                                     name="e_sbB")
                if engine == "act":
                    nc.scalar.activation(e_sb[:, :ncb * 128], slot[:, :ncb * 128],
                                         AF.Exp, scale=0.5)
                    track("act", svc_a)
                else:
                    nc.vector.tensor_scalar(
                        e_sb.bitcast(i32)[:, :ncb * 128], slot[:, :ncb * 128],
                        A_EXP, B_EXP, op0=OP.mult, op1=OP.add)
                    track("dve", svc_d)
                pend.append((ch, e_sb, run_idx, run_start))
                if len(pend) > AV_LAG:
                    flush_av(pend.pop(0))
                chunk_idx[0] += 1

            def flush_all_av():
                while pend:
                    flush_av(pend.pop(0))

            # ---------- tail per query run segment (staged pieces) -----
            tail_queue = []

            def emit_tail_piece():
                if tail_queue:
                    tail_queue.pop(0)()
                    return True
                return False

            def emit_tail(run_idx, q_start, width, defer=True):
                """Queue normalize/transpose/proj/BN/unpool/DMA stages for
                [q_start, q_start+width) so PE pieces interleave with the
                next run's chunks instead of blocking the in-order PE queue."""
                nqb = width // 128
                a = acc[run_idx]
                av = a.rearrange("p (q h e) -> p q h e", h=16, e=8)
                t_box = {}

                def norm_stage():
                    if debug_taps and q_start == 0:
                        accdump = sg1.tile([128, 160], f32)
                        nc.scalar.copy(
                            accdump.rearrange("p (q h e) -> p q h e", h=16, e=5),
                            av[:, 0:2, :, 0:5])
                        accd = nc.dram_tensor("tap_acc0", [128, 160],
                                              mybir.dt.float32,
                                              kind="ExternalOutput").ap()
                        nc.sync.dma_start(out=accd, in_=accdump)
                    track("dve", 300 + 300 * nqb)
                    track("act", 400 * nqb)
                    drs = dr[:, 0:16 * nqb]
                    nc.vector.reciprocal(
                        drs, av[:, 0:nqb, :, 4].rearrange("p q h -> p (q h)"))
                    onr = o_norm.rearrange("p (q h e) -> p q h e", h=16, e=4)
                    drb = drs.rearrange("p (q h) -> p q h", h=16).rearrange(
                        "p q (h x) -> p q h x", x=1).broadcast_to(
                        [128, nqb, 16, 4])
                    nc.vector.tensor_tensor(onr[:, 0:nqb], av[:, 0:nqb, :, 0:4],
                                            drb, op=OP.mult)

                def transpose_stage():
                    t_ps = slots.tile([128, 1024], f32, tag="slotA", bufs=3,
                                      name="t_ps")
                    t_box["t"] = t_ps
                    for qb in range(nqb):
                        nc.tensor.matmul(
                            t_ps[0:64, qb * 128:(qb + 1) * 128],
                            o_norm[:, qb * 64:(qb + 1) * 64], ident_sb,
                            is_transpose=True, start=True, stop=True)
                    qsl = slice(q_start, q_start + width)
                    nc.scalar.copy(o_normT[:, qsl], t_ps[0:64, 0:width])

                def proj_stage():
                    qsl = slice(q_start, q_start + width)
                    p_ps = slots.tile([128, 1024], f32, tag="slotA", bufs=3,
                                      name="p_ps")
                    nc.tensor.matmul(p_ps[0:64, 0:width], wproj_sb,
                                     o_normT[:, qsl],
                                     start=True, stop=True)
                    nc.scalar.activation(y[:, qsl], p_ps[0:64, 0:width],
                                         AF.Identity,
                                         bias=bnb_sb, scale=bns_sb)

                def out_stage():
                    sl = slice(q_start // 32, (q_start + width) // 32)
                    for p in range(4):
                        eng = nc.vector if p % 2 else nc.gpsimd
                        eng.tensor_tensor(
                            outr[:, sl, p // 2, :, p % 2], yr[:, sl],
                            eqr[p][:, sl], op=OP.mult)
                    nc.sync.dma_start(
                        out=out_d[:, q_start * 4:(q_start + width) * 4],
                        in_=out_sb[:, q_start * 4:(q_start + width) * 4])

                norm_stage()
                stages = [transpose_stage, proj_stage, out_stage]
                if defer:
                    tail_queue.extend(stages)
                else:
                    for s in stages:
                        s()

            # ---------- schedule ----------
            emit_prep_item(0, 0)
            emit_prep_item(1, 0)
            emit_v_item(0)

            ring = [8, 8, 8, 4]
            ring_pos = [0]

            def make_chunks(blocks):
                out = []
                i = 0
                while i < len(blocks):
                    n = ring[ring_pos[0] % 4]
                    out.append(blocks[i:i + n])
                    i += n
                    ring_pos[0] += 1
                return out

            runs = [(0, 256), (256, 256), (512, 256), (768, 128), (896, 128)]
            for run_idx, (q_start, width) in enumerate(runs):
                blocks = [(h, kt, qs) for kt in range(NKT)
                          for h in range(NUM_HEADS)
                          for qs in range(q_start, q_start + width, 128)]
                chunks = make_chunks(blocks)
                acc[run_idx] = accp.tile([128, 256], f32, tag="acc",
                                         name=f"a{run_idx}")
                nc.vector.memset(acc[run_idx][:, 0:width], 0.0)
                track("dve", 200 + width // 2)
                for ci, ch in enumerate(chunks):
                    emit_score_chunk(ch, run_idx, q_start)
                    if run_idx == 0:
                        if ci == 2:
                            emit_prep_item(0, 1)
                        elif ci == 3:
                            emit_prep_item(1, 1)
                        elif ci == 4:
                            emit_maxpool_quarter(2, pool_only=True)
                            emit_maxpool_quarter(3, pool_only=True)
                        elif ci == 5:
                            emit_v_item(1)
                        elif ci == 8:
                            emit_prep_item(0, 2)
                        elif ci == 9:
                            emit_prep_item(1, 2)
                        elif ci == 10:
                            emit_v_item(2)
                        elif ci == 12:
                            emit_prep_item(0, 3)
                        elif ci == 13:
                            emit_prep_item(1, 3)
                        elif ci == 14:
                            emit_v_item(3)
                        elif ci >= 16:
                            if not emit_tail_piece():
                                emit_mask_step()
                    elif ci % 2 == 0:
                        if not emit_tail_piece():
                            emit_mask_step()
                    else:
                        emit_mask_step()
                flush_all_av()
                emit_tail(run_idx, q_start, width,
                          defer=(run_idx < len(runs) - 1))

            if debug_taps:
                taps = {"pooled": pooled, "qtp": qtp, "ktp": ktp,
                        "onormT": o_normT, "y": y,
                        "mask0": eq[0], "mask3": eq[3], "dr": dr}
                for nm, t in taps.items():
                    d = nc.dram_tensor(f"tap_{nm}", list(t.shape),
                                       mybir.dt.float32, kind="ExternalOutput").ap()
                    nc.sync.dma_start(out=d, in_=t.bitcast(f32))
                vtd = nc.dram_tensor("tap_vt0", [128, 80], mybir.dt.float32,
                                     kind="ExternalOutput").ap()
                vtf = sg1.tile([128, 80], f32)
                nc.vector.tensor_copy(vtf, vt[0].rearrange("p h e -> p (h e)"))
                nc.sync.dma_start(out=vtd, in_=vtf)

    nc.compile()
    return nc


def _host_inputs(x, w_qkv, w_proj, gamma, beta, bn_mean, bn_var):
    """Build the per-core input maps (host-side packing)."""
    import ml_dtypes

    wv = np.ascontiguousarray(w_qkv[:, 128:192], dtype=np.float32)
    wb = np.zeros((128, 322), np.float32)
    wb[0:64, 0:128] = w_qkv[:, 0:128]                # dense wq | wk
    inv = gamma / np.sqrt(bn_var + BN_EPS)
    wb[0:64, 128] = inv.astype(np.float32)
    wb[0:64, 129] = (beta - bn_mean * inv).astype(np.float32)
    wb[0:64, 130:194] = wv
    wb[:, 194:322] = np.eye(128, dtype=np.float32)
    wr = np.ascontiguousarray(w_proj, dtype=np.float32)
    shared = {"wb": wb, "wr": wr}
    in_maps = []
    for b in range(B):
        m = dict(shared)
        m["x"] = np.ascontiguousarray(
            np.asarray(x)[b].reshape(DIM, H * W), dtype=np.float32)
        in_maps.append(m)
    return in_maps


def kernel(x, w_qkv, w_proj, gamma, beta, bn_mean, bn_var):
    from concourse import bass_utils

    if "nc" not in _CACHE:
        _CACHE["nc"] = _build_program()
    nc = _CACHE["nc"]
    in_maps = _host_inputs(
        np.asarray(x), np.asarray(w_qkv), np.asarray(w_proj),
        np.asarray(gamma), np.asarray(beta),
        np.asarray(bn_mean), np.asarray(bn_var))
    res = bass_utils.run_bass_kernel_spmd(nc, in_maps, core_ids=list(range(B)))
    out = np.stack([res.results[b]["out"].reshape(DIM, H, W) for b in range(B)])
    return out.astype(np.float32)


# revision 24
# speedup vs baseline: 1.0705x; 1.0705x over previous
"""Trainium2 Bass kernel for nn_D_GA_1812476199112 (maxpool -> 16-head
attention over 1024 tokens -> proj -> batchnorm -> maxunpool).

Sharding: data-parallel over batch B=8, one batch element per NeuronCore.
Everything is local per core; no collectives.

Per-core pipeline (channels-on-partitions layout [C=64, N=1024]):
  1. MaxPool2d(2,2) via strided max ops pipelined with the x DMA in four
     quarters (DVE+Pool); argmax becomes first-match masks computed
     per-quarter on GpSimd during attention idle.
  2. Q^T/K^T strip-packed (head h=4sg+c at partitions 32c..32c+4) from
     host-permuted padded weights, laid out quarter-major so one copy
     drains two supergroups; pooled is f32r so prep runs single-pass.
     Prep flows through the same PSUM slot rotation as score chunks.
  3. Score matmuls compute S^T [keys, queries] (K=4, row strips) into
     [128, 1024] PSUM slots (3-slot rotation), 4 blocks of 256 queries.
  4. Softmax exp splits across two engines: ACT runs real Exp (scale 0.5
     folded); DVE runs a Schraudolph bit-trick exp (i32 = A*s + B written
     through an int32 bitcast of the f32r e-tile; one tensor_scalar per
     chunk; end-to-end error ~2e-3, verified bit-exact vs numpy on HW).
  5. AV uses the EXP TILE AS THE STATIONARY operand: out[q,5] accumulates
     e_chunk[keys, qb*128..]^T @ v~[keys, 5h] over key tiles (ones col of
     v~ gives softmax denominators for free).  5-col bf16-moving matmuls
     cost ~2ns each, vs 131k PE columns for the moving-e formulation.
     AV lags the slot rotation by the slot count so it never stalls PE.
  6. Work runs in 4 query-quarter passes (256 q each) so the tail
     (normalize, PE transpose via identity, proj, BN Identity, unpool,
     DMA out) of each quarter hides under the next quarter's attention.
"""
import numpy as np

DIM = 64
HEAD_DIM = 4
NUM_HEADS = 16
B = 8
H = W = 64
HP = WP = 32
N = HP * WP          # 1024 tokens
NKT = 8              # key tiles of 128
QW = 256             # queries per quarter-run
BN_EPS = 1e-5

# Schraudolph exp constants (folds the softmax scale 0.5):
#   e^(0.5*s) ~= bitcast_f32(int32(A_EXP*s + B_EXP))
A_EXP = float(0.5 * np.log2(np.e) * (1 << 23))
B_EXP = float(127 * (1 << 23) - 486411)

_CACHE = {}

# ---- tuning knobs ----
N_WARM = 12          # PE warmup dummies at t=0
N_WARM2 = 6          # bridge dummies before prep
AV_LAG = 4           # chunks between scores and their AV flush (= slot count)
ACT_FRAC = 0.54      # fraction of exp chunks on ACT (rest DVE)
VT_ON_ACT = True     # v~ PSUM->bf16 copies on ACT instead of DVE


def _build_program(debug_taps=False):
    import concourse.bass as bass
    import concourse.mybir as mybir
    import concourse.tile as tile
    from concourse import bacc

    f32 = mybir.dt.float32
    f32r = mybir.dt.float32r
    bf16 = mybir.dt.bfloat16
    i32 = mybir.dt.int32
    AF = mybir.ActivationFunctionType
    OP = mybir.AluOpType

    nc = bacc.Bacc("TRN2", debug=False)

    x_d = nc.dram_tensor("x", [DIM, H * W], f32, kind="ExternalInput").ap()
    wb_d = nc.dram_tensor("wb", [128, 290], f32, kind="ExternalInput").ap()
    wr_d = nc.dram_tensor("wr", [64, 64], f32r, kind="ExternalInput").ap()
    out_d = nc.dram_tensor("out", [DIM, H * W], f32, kind="ExternalOutput").ap()

    with tile.TileContext(nc) as tc:
        with (
            tc.tile_pool(name="singles", bufs=1) as sg1,
            tc.tile_pool(name="expp", bufs=5) as expp,
            tc.tile_pool(name="slots", bufs=3, space="PSUM") as slots,
            tc.tile_pool(name="accp", bufs=1, space="PSUM") as accp,
        ):
            # hoist the ACT exp-table load to t=0 via a dummy exp
            warm = sg1.tile([1, 1], f32)
            nc.vector.memset(warm, 0.0)
            nc.scalar.activation(warm, warm, AF.Exp)

            # ---------- loads ----------
            x_sb = sg1.tile([DIM, H * W], f32)
            xr = x_sb.rearrange("p (i ti j tj) -> p i ti j tj", ti=2, tj=2, j=WP)
            wb_sb = sg1.tile([128, 290], f32)
            wr_sb = sg1.tile([64, 64], f32r)
            nc.sync.dma_start(out=wr_sb, in_=wr_d)
            nc.sync.dma_start(out=x_sb[:, 0:1024], in_=x_d[:, 0:1024])
            nc.sync.dma_start(out=wb_sb, in_=wb_d)
            for qq in (1, 2, 3):
                nc.sync.dma_start(out=x_sb[:, qq * 1024:(qq + 1) * 1024],
                                  in_=x_d[:, qq * 1024:(qq + 1) * 1024])
            wproj_sb = wr_sb
            bns_sb = wb_sb[0:64, 128:129]
            bnb_sb = wb_sb[0:64, 129:130]
            wv_sb = wb_sb[0:64, 130:162].bitcast(bf16)      # [64, 64] bf16
            ident_sb = wb_sb[:, 162:290]                    # [128, 128] f32

            # strip-pack dense wq/wk on device: head h=4sg+c lives at cols
            # 32c..32c+4 of qkp's per-sg 128-col block (q), +512 for k
            qkp = sg1.tile([64, 1024], bf16)
            nc.gpsimd.memset(qkp, 0.0)
            for sg in range(4):
                for qk in range(2):
                    dst = qkp[:, qk * 512 + sg * 128:qk * 512 + (sg + 1) * 128]
                    nc.gpsimd.tensor_copy(
                        dst.rearrange("p (c x d) -> p c x d", c=4, x=8)[:, :, 0, 0:4],
                        wb_sb[0:64, 64 * qk + 16 * sg:64 * qk + 16 * (sg + 1)]
                        .rearrange("p (c d) -> p c d", d=4))
            wqp_sb = [qkp[:, 128 * sg:128 * sg + 128] for sg in range(4)]
            wkp_sb = [qkp[:, 512 + 128 * sg:512 + 128 * sg + 128]
                      for sg in range(4)]

            # PE warmup dummies (keep HAM clock hot through the head phase)
            dummy_bf = sg1.tile([64, 512], bf16)
            nc.vector.memset(dummy_bf, 1.0)

            def emit_dummies(k):
                for _ in range(k):
                    d_ps = slots.tile([128, 512], f32, tag="slotB", bufs=1,
                                      name="d_ps")
                    nc.tensor.matmul(d_ps[0:64, :], dummy_bf[:, 0:64], dummy_bf,
                                     start=True, stop=True)

            emit_dummies(N_WARM)

            # ---------- maxpool (per x-quarter) ----------
            pooled = sg1.tile([DIM, N], f32)
            pooled_bf = sg1.tile([DIM, N], bf16)
            m01 = sg1.tile([DIM, N], f32)
            m23 = sg1.tile([DIM, N], f32)
            m01r = m01.rearrange("p (i j) -> p i j", j=WP)
            m23r = m23.rearrange("p (i j) -> p i j", j=WP)
            pooledr = pooled.rearrange("p (i j) -> p i j", j=WP)
            v = [xr[:, :, 0, :, 0], xr[:, :, 0, :, 1],
                 xr[:, :, 1, :, 0], xr[:, :, 1, :, 1]]

            def emit_maxpool_quarter(qq, pool_only=False):
                sl = slice(qq * 8, (qq + 1) * 8)
                e1 = nc.gpsimd if pool_only else nc.vector
                nc.gpsimd.tensor_tensor(m23r[:, sl], v[2][:, sl], v[3][:, sl],
                                        op=OP.max)
                e1.tensor_tensor(m01r[:, sl], v[0][:, sl], v[1][:, sl], op=OP.max)
                e1.tensor_tensor(pooledr[:, sl], m01r[:, sl], m23r[:, sl],
                                 op=OP.max)
                nc.gpsimd.tensor_copy(
                    pooled_bf[:, qq * 256:(qq + 1) * 256],
                    pooled[:, qq * 256:(qq + 1) * 256])

            emit_maxpool_quarter(0)
            emit_maxpool_quarter(1)
            emit_dummies(N_WARM2)

            # ---------- persistent SBUF tiles ----------
            # qk layout (bf16): [128, (qq)(pair)(q512|k512)] where the 512
            # is 2 supergroups x 256 quarter-queries/keys
            qk = sg1.tile([128, 8192], bf16)
            vt = [sg1.tile([128, NUM_HEADS, 5], bf16, tag=f"vt{kt}", name=f"vt{kt}")
                  for kt in range(NKT)]
            ones16 = sg1.tile([128, 16], bf16)
            nc.vector.memset(ones16, 1.0)
            o_norm = sg1.tile([128, 128], f32)     # [q2, h16, d4]
            o_normT = sg1.tile([64, N], f32r)
            dr = sg1.tile([128, 32], f32)
            y = sg1.tile([DIM, N], f32)
            yr = y.rearrange("p (i j) -> p i j", j=WP)
            out_sb = sg1.tile([DIM, H * W], f32)
            outr = out_sb.rearrange("p (i ti j tj) -> p i ti j tj",
                                    ti=2, tj=2, j=WP)

            def q_slice(sg, qs):
                base = ((qs // 256) * 2048 + (sg // 2) * 1024
                        + (sg % 2) * 256 + ((qs // 128) % 2) * 128)
                return slice(base, base + 128)

            def k_slice(sg, kt):
                base = ((kt // 2) * 2048 + (sg // 2) * 1024 + 512
                        + (sg % 2) * 256 + (kt % 2) * 128)
                return slice(base, base + 128)

            # ---------- masks (GpSimd filler, per i-quarter chains) --------
            eq = [sg1.tile([DIM, N], f32, tag=f"eq{p}", name=f"eq{p}")
                  for p in range(4)]
            nf = [sg1.tile([DIM, N], f32, tag=f"nf{p}", name=f"nf{p}")
                  for p in range(2)]
            eqr = [t.rearrange("p (i j) -> p i j", j=WP) for t in eq]
            nfr = [t.rearrange("p (i j) -> p i j", j=WP) for t in nf]
            mask_steps = []
            for mq in range(4):
                msl = slice(mq * 8, (mq + 1) * 8)
                g = nc.gpsimd
                mask_steps += [
                    lambda sl=msl, g=g: g.tensor_tensor(
                        eqr[0][:, sl], v[0][:, sl], pooledr[:, sl], op=OP.is_equal),
                    lambda sl=msl, g=g: g.tensor_tensor(
                        eqr[1][:, sl], v[1][:, sl], pooledr[:, sl], op=OP.is_equal),
                    lambda sl=msl, g=g: g.tensor_tensor(
                        eqr[2][:, sl], v[2][:, sl], pooledr[:, sl], op=OP.is_equal),
                    lambda sl=msl, g=g: g.tensor_tensor(
                        eqr[3][:, sl], v[3][:, sl], pooledr[:, sl], op=OP.is_equal),
                    lambda sl=msl, g=g: g.tensor_scalar(
                        nfr[0][:, sl], eqr[0][:, sl], -1.0, 1.0,
                        op0=OP.mult, op1=OP.add),
                    lambda sl=msl, g=g: g.tensor_tensor(
                        eqr[1][:, sl], eqr[1][:, sl], nfr[0][:, sl], op=OP.mult),
                    lambda sl=msl, g=g: g.tensor_tensor(
                        nfr[1][:, sl], nfr[0][:, sl], eqr[1][:, sl], op=OP.subtract),
                    lambda sl=msl, g=g: g.tensor_tensor(
                        eqr[2][:, sl], eqr[2][:, sl], nfr[1][:, sl], op=OP.mult),
                    lambda sl=msl, g=g: g.tensor_tensor(
                        nfr[0][:, sl], nfr[1][:, sl], eqr[2][:, sl], op=OP.subtract),
                    lambda sl=msl, g=g: g.tensor_tensor(
                        eqr[3][:, sl], eqr[3][:, sl], nfr[0][:, sl], op=OP.mult),
                ]
            mask_done = [0]

            def emit_mask_step():
                if mask_done[0] >= len(mask_steps):
                    return False
                mask_steps[mask_done[0]]()
                mask_done[0] += 1
                return True

            # ---------- pipelined work items over the PSUM slot rotation ----
            pend = []
            av_count = {}
            acc = [None] * 5
            est = {"act": 0.0, "dve": 0.0}

            def track(engine, ns):
                est[engine] += ns

            def emit_prep_item(pair, qq):
                """qt+kt prep for supergroups {2*pair, 2*pair+1}, quarter qq;
                one merged bf16 copy drains the slot."""
                psl = slice(qq * QW, (qq + 1) * QW)
                slot = slots.tile([128, 1024], f32, tag="slotA", bufs=3,
                                  name="slotP")
                for j, sg in enumerate((2 * pair, 2 * pair + 1)):
                    nc.tensor.matmul(slot[:, j * QW:(j + 1) * QW],
                                     wqp_sb[sg], pooled_bf[:, psl],
                                     start=True, stop=True)
                    nc.tensor.matmul(slot[:, 512 + j * QW:512 + (j + 1) * QW],
                                     wkp_sb[sg], pooled_bf[:, psl],
                                     start=True, stop=True)
                base = qq * 2048 + pair * 1024
                if est["act"] + 1040 <= est["dve"] + 1190:
                    nc.scalar.copy(qk[:, base:base + 1024], slot)
                    track("act", 1040)
                else:
                    nc.vector.tensor_copy(qk[:, base:base + 1024], slot)
                    track("dve", 1190)

            def emit_v_item(qq):
                """V~ for key tiles 2*qq, 2*qq+1 via the small PSUM slot."""
                v_ps = slots.tile([128, 512], f32, tag="slotB", bufs=1,
                                  name="v_ps")
                for j in range(2):
                    kt = 2 * qq + j
                    nc.tensor.matmul(v_ps[:, j * 64:(j + 1) * 64],
                                     pooled_bf[:, kt * 128:(kt + 1) * 128],
                                     wv_sb, start=True, stop=True)
                cp = nc.scalar.copy if VT_ON_ACT else nc.vector.tensor_copy
                track("act" if VT_ON_ACT else "dve", 520)
                for j in range(2):
                    kt = 2 * qq + j
                    cp(vt[kt][:, :, 0:4],
                       v_ps[:, j * 64:(j + 1) * 64].rearrange(
                           "p (h e) -> p h e", e=4))
                    nc.gpsimd.tensor_copy(
                        vt[kt].rearrange("p h e -> p (h e)")[:, 4::5], ones16)

            def flush_av(payload):
                ch, e_sb, run_idx, run_start = payload
                for i, (h, kt, qs) in enumerate(ch):
                    qbl = (qs - run_start) // 128
                    key = (qs, h)
                    cnt = av_count.get(key, 0)
                    av_count[key] = cnt + 1
                    nc.tensor.matmul(
                        acc[run_idx][:, qbl * 128 + h * 8:qbl * 128 + h * 8 + 5],
                        e_sb[:, i * 128:(i + 1) * 128],
                        vt[kt][:, h, :],
                        start=False, stop=(cnt == NKT - 1),
                        skip_group_check=True)

            chunk_idx = [0]

            def emit_score_chunk(ch, run_idx, run_start, engine=None):
                ncb = len(ch)
                svc_a = 107 * ncb + 220
                svc_d = 133 * ncb + 160
                if engine is None:
                    engine = ("act" if est["act"] + svc_a <= est["dve"] + svc_d
                              else "dve")
                if ncb > 4:
                    slot = slots.tile([128, 1024], f32, tag="slotA", bufs=3,
                                      name="slotA")
                else:
                    slot = slots.tile([128, 512], f32, tag="slotB", bufs=1,
                                      name="slotB")
                for i, (h, kt, qs) in enumerate(ch):
                    sg, c = h // 4, h % 4
                    nc.tensor.matmul(
                        slot[:, i * 128:(i + 1) * 128],
                        qk[32 * c:32 * c + 4, k_slice(sg, kt)],
                        qk[32 * c:32 * c + 4, q_slice(sg, qs)],
                        start=True, stop=True,
                        tile_position=(32 * c, 0))
                if ncb > 4:
                    e_sb = expp.tile([128, 1024], f32r, tag="exp", bufs=5,
                                     name="e_sb")
                else:
                    e_sb = expp.tile([128, 512], f32r, tag="expB", bufs=2,
                                     name="e_sbB")
                if engine == "act":
                    nc.scalar.activation(e_sb[:, :ncb * 128], slot[:, :ncb * 128],
                                         AF.Exp, scale=0.5)
                    track("act", svc_a)
                else:
                    nc.vector.tensor_scalar(
                        e_sb.bitcast(i32)[:, :ncb * 128], slot[:, :ncb * 128],
                        A_EXP, B_EXP, op0=OP.mult, op1=OP.add)
                    track("dve", svc_d)
                pend.append((ch, e_sb, run_idx, run_start))
                if len(pend) > AV_LAG:
                    flush_av(pend.pop(0))
                chunk_idx[0] += 1

            def flush_all_av():
                while pend:
                    flush_av(pend.pop(0))

            # ---------- tail per query run segment (staged pieces) -----
            tail_queue = []

            def emit_tail_piece():
                if tail_queue:
                    tail_queue.pop(0)()
                    return True
                return False

            def emit_tail(run_idx, q_start, width, defer=True):
                """Queue normalize/transpose/proj/BN/unpool/DMA stages for
                [q_start, q_start+width) so PE pieces interleave with the
                next run's chunks instead of blocking the in-order PE queue."""
                nqb = width // 128
                a = acc[run_idx]
                av = a.rearrange("p (q h e) -> p q h e", h=16, e=8)
                t_box = {}

                def norm_stage():
                    if debug_taps and q_start == 0:
                        accdump = sg1.tile([128, 160], f32)
                        nc.scalar.copy(
                            accdump.rearrange("p (q h e) -> p q h e", h=16, e=5),
                            av[:, 0:2, :, 0:5])
                        accd = nc.dram_tensor("tap_acc0", [128, 160],
                                              mybir.dt.float32,
                                              kind="ExternalOutput").ap()
                        nc.sync.dma_start(out=accd, in_=accdump)
                    track("dve", 300 + 300 * nqb)
                    track("act", 400 * nqb)
                    drs = dr[:, 0:16 * nqb]
                    nc.vector.reciprocal(
                        drs, av[:, 0:nqb, :, 4].rearrange("p q h -> p (q h)"))
                    onr = o_norm.rearrange("p (q h e) -> p q h e", h=16, e=4)
                    drb = drs.rearrange("p (q h) -> p q h", h=16).rearrange(
                        "p q (h x) -> p q h x", x=1).broadcast_to(
                        [128, nqb, 16, 4])
                    nc.vector.tensor_tensor(onr[:, 0:nqb], av[:, 0:nqb, :, 0:4],
                                            drb, op=OP.mult)

                def transpose_stage():
                    t_ps = slots.tile([128, 1024], f32, tag="slotA", bufs=3,
                                      name="t_ps")
                    t_box["t"] = t_ps
                    for qb in range(nqb):
                        nc.tensor.matmul(
                            t_ps[0:64, qb * 128:(qb + 1) * 128],
                            o_norm[:, qb * 64:(qb + 1) * 64], ident_sb,
                            is_transpose=True, start=True, stop=True)
                    qsl = slice(q_start, q_start + width)
                    nc.scalar.copy(o_normT[:, qsl], t_ps[0:64, 0:width])

                def proj_stage():
                    qsl = slice(q_start, q_start + width)
                    p_ps = slots.tile([128, 1024], f32, tag="slotA", bufs=3,
                                      name="p_ps")
                    nc.tensor.matmul(p_ps[0:64, 0:width], wproj_sb,
                                     o_normT[:, qsl],
                                     start=True, stop=True)
                    nc.scalar.activation(y[:, qsl], p_ps[0:64, 0:width],
                                         AF.Identity,
                                         bias=bnb_sb, scale=bns_sb)

                def out_stage():
                    sl = slice(q_start // 32, (q_start + width) // 32)
                    for p in range(4):
                        eng = nc.vector if p % 2 else nc.gpsimd
                        eng.tensor_tensor(
                            outr[:, sl, p // 2, :, p % 2], yr[:, sl],
                            eqr[p][:, sl], op=OP.mult)
                    nc.sync.dma_start(
                        out=out_d[:, q_start * 4:(q_start + width) * 4],
                        in_=out_sb[:, q_start * 4:(q_start + width) * 4])

                norm_stage()
                stages = [transpose_stage, proj_stage, out_stage]
                if defer:
                    tail_queue.extend(stages)
                else:
                    for s in stages:
                        s()

            # ---------- schedule ----------
            emit_prep_item(0, 0)
            emit_prep_item(1, 0)
            emit_v_item(0)

            ring = [8, 8, 8, 4]
            ring_pos = [0]

            def make_chunks(blocks):
                out = []
                i = 0
                while i < len(blocks):
                    n = ring[ring_pos[0] % 4]
                    out.append(blocks[i:i + n])
                    i += n
                    ring_pos[0] += 1
                return out

            runs = [(0, 256), (256, 256), (512, 256), (768, 128), (896, 128)]
            for run_idx, (q_start, width) in enumerate(runs):
                blocks = [(h, kt, qs) for kt in range(NKT)
                          for h in range(NUM_HEADS)
                          for qs in range(q_start, q_start + width, 128)]
                chunks = make_chunks(blocks)
                acc[run_idx] = accp.tile([128, 256], f32, tag="acc",
                                         name=f"a{run_idx}")
                nc.vector.memset(acc[run_idx][:, 0:width], 0.0)
                track("dve", 200 + width // 2)
                for ci, ch in enumerate(chunks):
                    emit_score_chunk(ch, run_idx, q_start)
                    if run_idx == 0:
                        if ci == 2:
                            emit_prep_item(0, 1)
                        elif ci == 3:
                            emit_prep_item(1, 1)
                        elif ci == 4:
                            emit_maxpool_quarter(2, pool_only=True)
                            emit_maxpool_quarter(3, pool_only=True)
                        elif ci == 5:
                            emit_v_item(1)
                        elif ci == 8:
                            emit_prep_item(0, 2)
                        elif ci == 9:
                            emit_prep_item(1, 2)
                        elif ci == 10:
                            emit_v_item(2)
                        elif ci == 12:
                            emit_prep_item(0, 3)
                        elif ci == 13:
                            emit_prep_item(1, 3)
                        elif ci == 14:
                            emit_v_item(3)
                        elif ci >= 16:
                            if not emit_tail_piece():
                                emit_mask_step()
                    elif ci % 2 == 0:
                        if not emit_tail_piece():
                            emit_mask_step()
                    else:
                        emit_mask_step()
                flush_all_av()
                emit_tail(run_idx, q_start, width,
                          defer=(run_idx < len(runs) - 1))

            if debug_taps:
                taps = {"pooled": pooled, "qtp": qtp, "ktp": ktp,
                        "onormT": o_normT, "y": y,
                        "mask0": eq[0], "mask3": eq[3], "dr": dr}
                for nm, t in taps.items():
                    d = nc.dram_tensor(f"tap_{nm}", list(t.shape),
                                       mybir.dt.float32, kind="ExternalOutput").ap()
                    nc.sync.dma_start(out=d, in_=t.bitcast(f32))
                vtd = nc.dram_tensor("tap_vt0", [128, 80], mybir.dt.float32,
                                     kind="ExternalOutput").ap()
                vtf = sg1.tile([128, 80], f32)
                nc.vector.tensor_copy(vtf, vt[0].rearrange("p h e -> p (h e)"))
                nc.sync.dma_start(out=vtd, in_=vtf)

    nc.compile()
    return nc


def _host_inputs(x, w_qkv, w_proj, gamma, beta, bn_mean, bn_var):
    """Build the per-core input maps (host-side packing)."""
    import ml_dtypes

    wv = np.ascontiguousarray(w_qkv[:, 128:192], dtype=np.float32)
    wb = np.zeros((128, 290), np.float32)
    wb[0:64, 0:128] = w_qkv[:, 0:128]                # dense wq | wk
    inv = gamma / np.sqrt(bn_var + BN_EPS)
    wb[0:64, 128] = inv.astype(np.float32)
    wb[0:64, 129] = (beta - bn_mean * inv).astype(np.float32)
    wv_bf = wv.astype(ml_dtypes.bfloat16)            # [64, 64] bf16
    wb[0:64, 130:162] = wv_bf.view(np.float32)
    wb[:, 162:290] = np.eye(128, dtype=np.float32)
    wr = np.ascontiguousarray(w_proj, dtype=np.float32)
    shared = {"wb": wb, "wr": wr}
    in_maps = []
    for b in range(B):
        m = dict(shared)
        m["x"] = np.ascontiguousarray(
            np.asarray(x)[b].reshape(DIM, H * W), dtype=np.float32)
        in_maps.append(m)
    return in_maps


def kernel(x, w_qkv, w_proj, gamma, beta, bn_mean, bn_var):
    from concourse import bass_utils

    if "nc" not in _CACHE:
        _CACHE["nc"] = _build_program()
    nc = _CACHE["nc"]
    in_maps = _host_inputs(
        np.asarray(x), np.asarray(w_qkv), np.asarray(w_proj),
        np.asarray(gamma), np.asarray(beta),
        np.asarray(bn_mean), np.asarray(bn_var))
    res = bass_utils.run_bass_kernel_spmd(nc, in_maps, core_ids=list(range(B)))
    out = np.stack([res.results[b]["out"].reshape(DIM, H, W) for b in range(B)])
    return out.astype(np.float32)
